# revision 1
# baseline (speedup 1.0000x reference)
"""Causal self-attention (QAT fake-quant weights, RMS-normed q/k, RoPE, GQA)
on 8 Trainium2 NeuronCores.

Sharding: core c = b*4 + t  (b in {0,1} batch, t in {0..3} tensor-parallel).
Per core: 4 q-heads (t*4..t*4+3), 1 kv head (t), Wproj columns [512t, 512t+512).
Each core computes a full [D, S] transposed partial of the output projection;
the host transposes and sums the 4 TP partials per batch element.

Everything on-device is feature-major ("transposed"): activations [feat, seq].
 - projections:   qT = qWqT.T @ xT  (contraction over d on partitions)
 - scoresT[k,q]  = krotT_tile.T @ qrotT  -> exp -> probsT (SBUF, f32r)
 - PV:            yT += v_nat_tile.T @ probsT   (v natural = [s, hd])
 - softmax sums:  ones[128,1].T @ probsT -> [1, q] PSUM accumulation
 - out:           outT = qWPT.T @ (yT / sums)
RoPE rotate-half is a PE permutation matmul + DVE mul/adds; rms_norm sums of
squares are ones-matmuls over qT^2; gain and 1/sqrt(hd) fold into the rsqrt.
Softmax skips max-subtraction (scores bounded by gain*sqrt(hd) ~ 11.3).
Fake quant: round(W * (1/s)) * s with s = fp16(max|W_blk|/31) per 128-block,
rounding via the +1.5*2^23 magic-constant trick (RNE, matches jnp.round).
"""

import os
from contextlib import ExitStack

import numpy as np

import concourse.bass as bass
import concourse.bacc as bacc
import concourse.tile as tile
from concourse import mybir
from concourse.bass_utils import run_bass_kernel_spmd

F32 = mybir.dt.float32
F32R = mybir.dt.float32r
F16 = mybir.dt.float16

DIM = 2048
S = 2048
HD = 128
HL = 4            # local q heads per core
CL = HL * HD      # local head dims (proj contraction)
NB = DIM // 128   # 16 blocks of 128 along a full input-feature axis
MAGIC = float(1.5 * 2 ** 23)
INV31 = float(np.float32(1.0) / np.float32(31.0))
EPS = float(np.finfo(np.float32).eps)
F16_TINY = float(np.finfo(np.float16).tiny)

Alu = mybir.AluOpType
Act = mybir.ActivationFunctionType

_CACHE = {}


def _emit_quant_smalls(nc, pool, wn, nb, pfx):
    """wn [128, nb*128] natural weight tile -> (sf, rf): scale and 1/scale."""
    amax = pool.tile([128, nb], F32, tag=pfx + "am")
    nc.vector.tensor_reduce(
        amax[:], wn[:].rearrange("p (b c) -> p b c", c=128),
        axis=mybir.AxisListType.X, op=Alu.max, apply_absolute_value=True)
    s0 = pool.tile([128, nb], F32, tag=pfx + "s0")
    nc.vector.tensor_scalar(s0[:], amax[:], INV31, 1e-12, Alu.mult, Alu.max)
    s16 = pool.tile([128, nb], F16, tag=pfx + "s16")
    nc.vector.tensor_copy(s16[:], s0[:])
    s32 = pool.tile([128, nb], F32, tag=pfx + "s32")
    nc.vector.tensor_copy(s32[:], s16[:])
    sf = pool.tile([128, nb], F32, tag=pfx + "sf")
    nc.vector.tensor_scalar_max(sf[:], s32[:], F16_TINY)
    rf = pool.tile([128, nb], F32, tag=pfx + "rf")
    nc.vector.reciprocal(rf[:], sf[:])
    return sf, rf


def _emit_quant_apply(nc, wpool, wn, sf, rf, nb, qtag, ttag, mode="dve"):
    """qw = round(wn * rf) * sf blockwise, via magic-constant RNE round.
    Three whole-tile ops using zero-stride block-broadcast views.
    mode: dve | split (last mult on gpsimd) | gpsimd (all on gpsimd)."""
    qw = wpool.tile([128, nb * 128], F32, tag=qtag)
    tt = wpool.tile([128, nb * 128], F32, tag=ttag)
    rview = rf[:].broadcast_to([128, nb, 128])
    sview = sf[:].broadcast_to([128, nb, 128])
    w3 = wn[:].rearrange("p (b c) -> p b c", c=128)
    t3 = tt[:].rearrange("p (b c) -> p b c", c=128)
    q3 = qw[:].rearrange("p (b c) -> p b c", c=128)
    e1 = nc.gpsimd if mode == "gpsimd" else nc.vector
    e3 = nc.gpsimd if mode in ("gpsimd", "split") else nc.vector
    e1.tensor_mul(t3, w3, rview)
    e1.tensor_scalar(tt[:], tt[:], MAGIC, MAGIC, Alu.add, Alu.subtract)
    e3.tensor_mul(q3, t3, sview)
    return qw


def build_nc():
    nc = bacc.Bacc("TRN2")

    XT = nc.dram_tensor("XT", [DIM, S], F32R, kind="ExternalInput")
    WQ = nc.dram_tensor("WQ", [CL, DIM], F32, kind="ExternalInput")
    WK = nc.dram_tensor("WK", [HD, DIM], F32, kind="ExternalInput")
    WV = nc.dram_tensor("WV", [HD, DIM], F32, kind="ExternalInput")
    WP = nc.dram_tensor("WP", [DIM, CL], F32, kind="ExternalInput")
    COSW = nc.dram_tensor("COSW", [128, S], F32, kind="ExternalInput")
    SINW = nc.dram_tensor("SINW", [128, S], F32, kind="ExternalInput")
    IDENT = nc.dram_tensor("IDENT", [128, 128], F32, kind="ExternalInput")
    PSWAP = nc.dram_tensor("PSWAP", [128, 128], F32R, kind="ExternalInput")
    TRIM = nc.dram_tensor("TRIM", [128, 128], F32, kind="ExternalInput")
    ONESC = nc.dram_tensor("ONESC", [128, 1], F32R, kind="ExternalInput")
    AVEC = nc.dram_tensor("AVEC", [1, 8], F32, kind="ExternalInput")
    BVEC = nc.dram_tensor("BVEC", [1, 8], F32, kind="ExternalInput")

    OUT = nc.dram_tensor("OUT", [DIM, S], F32, kind="ExternalOutput")

    copy_flip = [0]

    def copy_out(dst, src):
        # alternate PSUM->SBUF copies between ACT and DVE
        if copy_flip[0] % 2 == 0:
            nc.scalar.copy(dst, src)
        else:
            nc.vector.tensor_copy(dst, src)
        copy_flip[0] += 1

    with tile.TileContext(nc) as tc, ExitStack() as octx:
        # ---------------- always-live pools ----------------
        pc = octx.enter_context(tc.tile_pool(name="consts", bufs=1))
        prow = octx.enter_context(tc.tile_pool(name="rows", bufs=3))
        pdram = octx.enter_context(tc.tile_pool(name="dram", bufs=1,
                                                space="DRAM"))
        ps = octx.enter_context(tc.tile_pool(name="ps", bufs=3, space="PSUM"))
        psacc = octx.enter_context(tc.tile_pool(name="psacc", bufs=2,
                                                space="PSUM"))
        psrow = octx.enter_context(tc.tile_pool(name="psrow", bufs=2,
                                                space="PSUM"))

        ident = pc.tile([128, 128], F32)
        pswap = pc.tile([128, 128], F32R)
        trim = pc.tile([128, 128], F32)
        onesc = pc.tile([128, 1], F32R)
        avec = pc.tile([1, 8], F32)
        bvec = pc.tile([1, 8], F32)
        nc.sync.dma_start(ident[:], IDENT[:, :])
        nc.sync.dma_start(pswap[:], PSWAP[:, :])
        nc.sync.dma_start(trim[:], TRIM[:, :])
        nc.sync.dma_start(onesc[:], ONESC[:, :])
        nc.sync.dma_start(avec[:], AVEC[:, :])
        nc.sync.dma_start(bvec[:], BVEC[:, :])

        # yT spilled to DRAM between attention and output projection
        ytd = [pdram.tile([128, S], F32R, tag=f"ytd{h}", name=f"ytd{h}") for h in range(HL)]

        # ============== P1: quantize Wq/Wk/Wv + transpose ==============
        # qwt lives until the end; its 16 [128,512] tag slots are reused
        # for the quantized Wproj tiles in P5.
        pq1 = octx.enter_context(tc.tile_pool(name="qwt", bufs=1))
        qWqT = [pq1.tile([128, CL], F32R, tag=f"qwq{d}", name=f"qwq{d}")
                for d in range(NB)]
        qWkT = [pq1.tile([128, 4, 128], F32R, tag=f"qwk{g}", name=f"qwk{g}")
                for g in range(4)]
        qWvT = [pq1.tile([128, 4, 128], F32R, tag=f"qwv{g}", name=f"qwv{g}")
                for g in range(4)]

        es1 = ExitStack()   # P1 working pools — close right after P1
        pw2 = es1.enter_context(tc.tile_pool(name="p1w2", bufs=2))
        pw4 = es1.enter_context(tc.tile_pool(name="p1w4", bufs=4))
        pws = es1.enter_context(tc.tile_pool(name="p1s", bufs=2))

        for W, dst in ((WK, qWkT), (WV, qWvT)):
            wn = pw2.tile([128, DIM], F32, tag="wnat")
            nc.sync.dma_start(wn[:], W[:, :])
            sf, rf = _emit_quant_smalls(nc, pws, wn, NB, "q")
            qw = _emit_quant_apply(nc, pw4, wn, sf, rf, NB, "wqq", "wtmp",
                                   mode="split")
            for g in range(4):
                pt = ps.tile([128, 512], F32, tag="mm")
                for j in range(4):
                    blk = 4 * g + j
                    nc.tensor.transpose(pt[:, j * 128:(j + 1) * 128],
                                        qw[:, blk * 128:(blk + 1) * 128],
                                        ident[:])
                copy_out(dst[g][:].rearrange("p a b -> p (a b)"), pt[:])

        # Wq: 4 natural row-tiles; keep the 4 qw tiles for batched transposes
        qwq = []
        for ot in range(4):
            wn = pw2.tile([128, DIM], F32, tag="wnat")
            nc.sync.dma_start(wn[:], WQ[ot * 128:(ot + 1) * 128, :])
            sf, rf = _emit_quant_smalls(nc, pws, wn, NB, "q")
            qwq.append(_emit_quant_apply(nc, pw4, wn, sf, rf, NB,
                                         "wqq", "wtmp", mode="split"))
        for blk in range(NB):
            pt = ps.tile([128, 512], F32, tag="mm")
            for ot in range(4):
                nc.tensor.transpose(pt[:, ot * 128:(ot + 1) * 128],
                                    qwq[ot][:, blk * 128:(blk + 1) * 128],
                                    ident[:])
            copy_out(qWqT[blk][:], pt[:])

        es1.close()

        # persistent attention operands (allocated after P1 pools freed)
        pp = octx.enter_context(tc.tile_pool(name="persist", bufs=1))
        qrot = [pp.tile([128, S], F32R, tag=f"qrot{h}", name=f"qrot{h}")
                for h in range(HL)]
        krot = pp.tile([128, S], F32R, tag="krot")
        vnat = pp.tile([128, NB, 128], F32R, tag="vnat")  # [s%128, s//128, hd]

        # ============== P2+P3 fused: projections + rms + rope =========
        es2 = ExitStack()
        px = es2.enter_context(tc.tile_pool(name="p2x", bufs=24))
        p2t = es2.enter_context(tc.tile_pool(name="p2t", bufs=2))
        p2c = es2.enter_context(tc.tile_pool(name="p2c", bufs=7))
        p2b = es2.enter_context(tc.tile_pool(name="p2b", bufs=2))

        for sc in range(4):
            ssl = slice(sc * 512, (sc + 1) * 512)
            xts = []
            for dt in range(NB):
                xt = px.tile([128, 512], F32R, tag="xt")
                nc.sync.dma_start(xt[:], XT[dt * 128:(dt + 1) * 128, ssl])
                xts.append(xt)
            cosw = p2t.tile([128, 512], F32, tag="cosw")
            sinw = p2t.tile([128, 512], F32, tag="sinw")
            nc.sync.dma_start(cosw[:], COSW[:, ssl])
            nc.sync.dma_start(sinw[:], SINW[:, ssl])

            raws = []
            for hm in range(HL + 1):  # 4 q heads then k
                pm = ps.tile([128, 512], F32, tag="mm")
                for dt in range(NB):
                    if hm < HL:
                        lhs = qWqT[dt][:, hm * 128:(hm + 1) * 128]
                    else:
                        lhs = qWkT[dt // 4][:, dt % 4, :]
                    nc.tensor.matmul(pm[:], lhs, xts[dt][:],
                                     start=(dt == 0), stop=(dt == NB - 1))
                raw = p2c.tile([128, 512], F32, tag="raw")
                nc.scalar.copy(raw[:], pm[:])
                raws.append(raw)
            # v projection; transpose to natural [s, hd]
            pm = ps.tile([128, 512], F32, tag="mm")
            for dt in range(NB):
                nc.tensor.matmul(pm[:], qWvT[dt // 4][:, dt % 4, :],
                                 xts[dt][:], start=(dt == 0),
                                 stop=(dt == NB - 1))
            vtr = p2c.tile([128, 512], F32, tag="raw")
            nc.scalar.copy(vtr[:], pm[:])
            pv = ps.tile([128, 512], F32, tag="mm")
            for j in range(4):
                nc.tensor.transpose(pv[:, j * 128:(j + 1) * 128],
                                    vtr[:, j * 128:(j + 1) * 128], ident[:])
            nc.vector.tensor_copy(
                vnat[:, 4 * sc:4 * sc + 4, :].rearrange("p a b -> p (a b)"),
                pv[:])

            for hm in range(HL + 1):
                raw = raws[hm]
                dst = qrot[hm] if hm < HL else krot
                qsq = p2t.tile([128, 512], F32R, tag="qsq")
                nc.vector.tensor_mul(qsq[:], raw[:], raw[:])
                ssp = psrow.tile([1, 512], F32, tag="row")
                nc.tensor.matmul(ssp[:], onesc[:], qsq[:],
                                 start=True, stop=True)
                rr = prow.tile([1, 512], F32, tag="prerow")
                nc.scalar.activation(rr[:], ssp[:], Act.Abs_reciprocal_sqrt,
                                     bias=bvec[0:1, hm:hm + 1],
                                     scale=avec[0:1, hm:hm + 1])
                rb = p2b.tile([128, 512], F32, tag="rb")
                nc.gpsimd.partition_broadcast(rb[:], rr[:])
                qn = p2t.tile([128, 512], F32R, tag="qn")
                nc.vector.tensor_mul(qn[:], raw[:], rb[:])
                # rope: dst = qn*cos + (PSWAP @ qn)*sin
                sw = ps.tile([128, 512], F32, tag="mm")
                nc.tensor.matmul(sw[:], pswap[:], qn[:],
                                 start=True, stop=True)
                u = p2t.tile([128, 512], F32, tag="u")
                nc.vector.tensor_mul(u[:], qn[:], cosw[:])
                w = p2t.tile([128, 512], F32, tag="w")
                nc.vector.tensor_mul(w[:], sw[:], sinw[:])
                nc.vector.tensor_add(dst[:, ssl], u[:], w[:])
        es2.close()

        # ============== P5 (Wproj quant) + P4 (attention) + P6 ==============
        es3 = ExitStack()
        p5w = es3.enter_context(tc.tile_pool(name="p5w", bufs=4))
        p5s = es3.enter_context(tc.tile_pool(name="p5s", bufs=2))
        pprob = es3.enter_context(tc.tile_pool(name="probs", bufs=8))
        pm4 = es3.enter_context(tc.tile_pool(name="p4m", bufs=2))

        # ---- P4: attention ----
        for h in range(HL):
            qr = qrot[h]
            for qc in range(4):
                qsl = slice(qc * 512, (qc + 1) * 512)
                yps = psacc.tile([128, 512], F32, tag="acc")
                sps = psrow.tile([1, 512], F32, tag="row")
                nkt = 4 * qc + 4
                for kt in range(nkt):
                    j = kt - 4 * qc
                    lo = 0 if j < 0 else 128 * j
                    scp = ps.tile([128, 512], F32, tag="mm")
                    nc.tensor.matmul(
                        scp[:, lo:], krot[:, kt * 128:(kt + 1) * 128],
                        qr[:, qc * 512 + lo:(qc + 1) * 512],
                        start=True, stop=True)
                    pr = pprob.tile([128, 512], F32R, tag="pr")
                    nc.scalar.activation(pr[:, lo:], scp[:, lo:], Act.Exp)
                    if j >= 0:
                        nc.vector.tensor_mul(pr[:, lo:lo + 128],
                                             pr[:, lo:lo + 128], trim[:])
                    nc.tensor.matmul(yps[:, lo:], vnat[:, kt, :], pr[:, lo:],
                                     start=(kt == 0), stop=(kt == nkt - 1))
                    nc.tensor.matmul(sps[0:1, lo:], onesc[:], pr[:, lo:],
                                     start=(kt == 0), stop=(kt == nkt - 1))
                scr = prow.tile([1, 512], F32, tag="prerow")
                rs = prow.tile([1, 512], F32, tag="prerow")
                nc.vector.reciprocal_approx_accurate(rs[:], sps[:], scr[:])
                rb2 = pm4.tile([128, 512], F32, tag="rb2")
                nc.gpsimd.partition_broadcast(rb2[:], rs[:])
                ya = pm4.tile([128, 512], F32, tag="ya")
                nc.vector.tensor_copy(ya[:], yps[:])
                yt = pm4.tile([128, 512], F32R, tag="yt")
                nc.vector.tensor_mul(yt[:], ya[:], rb2[:])
                nc.sync.dma_start(ytd[h][:, qsl], yt[:])

        qWPT = [pq1.tile([128, 512], F32R, tag=f"qwq{i}", name=f"qwp{i}")
                for i in range(16)]
        for og in range(4):  # groups of 4 o-tiles
            qwps = []
            for j in range(4):
                ot = 4 * og + j
                wn = p5w.tile([128, CL], F32, tag="wnat5")
                nc.sync.dma_start(wn[:], WP[ot * 128:(ot + 1) * 128, :])
                sf, rf = _emit_quant_smalls(nc, p5s, wn, 4, "p")
                qwps.append(_emit_quant_apply(nc, p5w, wn, sf, rf, 4,
                                              "wqp", "wtp", mode="gpsimd"))
            for blk in range(4):
                pt = ps.tile([128, 512], F32, tag="mm5", bufs=1)
                for j in range(4):
                    nc.tensor.transpose(
                        pt[:, j * 128:(j + 1) * 128],
                        qwps[j][:, blk * 128:(blk + 1) * 128], ident[:])
                copy_out(qWPT[4 * blk + og][:], pt[:])

        # ---- P6: output projection ----
        p6y = es3.enter_context(tc.tile_pool(name="p6y", bufs=8))
        p6o = es3.enter_context(tc.tile_pool(name="p6o", bufs=3))
        for qc in range(4):
            qsl = slice(qc * 512, (qc + 1) * 512)
            yts = []
            for hb in range(HL):
                yti = p6y.tile([128, 512], F32R, tag="ytin")
                nc.sync.dma_start(yti[:], ytd[hb][:, qsl])
                yts.append(yti)
            for ot in range(NB):
                op = ps.tile([128, 512], F32, tag="mm")
                for blk in range(4):
                    lhs = qWPT[4 * blk + ot // 4][:, (ot % 4) * 128:
                                                  (ot % 4 + 1) * 128]
                    nc.tensor.matmul(op[:], lhs, yts[blk][:],
                                     start=(blk == 0), stop=(blk == 3))
                ob = p6o.tile([128, 512], F32, tag="ob")
                copy_out(ob[:], op[:])
                nc.sync.dma_start(OUT[ot * 128:(ot + 1) * 128, qsl], ob[:])
        es3.close()

    nc.compile()
    return nc


# --------------------------------------------------------------------------
# host side
# --------------------------------------------------------------------------

def _host_consts():
    inv_freq = 1.0 / (10000.0 ** (np.arange(0, HD, 2, dtype=np.float32)
                                  / np.float32(HD)))
    freqs = np.outer(np.arange(S, dtype=np.float32),
                     inv_freq).astype(np.float32)       # [S, 64]
    cosT = np.cos(freqs).astype(np.float32).T           # [64, S]
    sinT = np.sin(freqs).astype(np.float32).T
    cosw = np.ascontiguousarray(np.concatenate([cosT, cosT], axis=0))
    sinw = np.ascontiguousarray(np.concatenate([sinT, -sinT], axis=0))
    ident = np.eye(128, dtype=np.float32)
    pswap = np.zeros((128, 128), dtype=np.float32)
    pswap[:64, 64:] = np.eye(64)
    pswap[64:, :64] = np.eye(64)
    trim = (np.arange(128)[:, None] <= np.arange(128)[None, :]) \
        .astype(np.float32)                             # allow k <= q
    onesc = np.ones((128, 1), dtype=np.float32)
    return cosw, sinw, ident, pswap, trim, onesc


def kernel(x, Wq, Wk, Wv, Wproj, q_gain):
    x = np.asarray(x, dtype=np.float32)
    Wq = np.asarray(Wq, dtype=np.float32)
    Wk = np.asarray(Wk, dtype=np.float32)
    Wv = np.asarray(Wv, dtype=np.float32)
    Wproj = np.asarray(Wproj, dtype=np.float32)
    q_gain = np.asarray(q_gain, dtype=np.float32)
    B = x.shape[0]

    if "nc" not in _CACHE:
        _CACHE["nc"] = build_nc()
    nc = _CACHE["nc"]

    cosw, sinw, ident, pswap, trim, onesc = _host_consts()

    in_maps = []
    for c in range(8):
        b, t = divmod(c, 4)
        g = q_gain[4 * t:4 * t + 4].astype(np.float64)
        avec = np.zeros((1, 8), dtype=np.float32)
        bvec = np.zeros((1, 8), dtype=np.float32)
        avec[0, :4] = (1.0 / g ** 2).astype(np.float32)
        avec[0, 4] = np.float32(1.0 / 128.0)
        bvec[0, :4] = (128.0 * EPS / g ** 2).astype(np.float32)
        bvec[0, 4] = np.float32(EPS)
        in_maps.append({
            "XT": np.ascontiguousarray(x[b].T),
            "WQ": np.ascontiguousarray(Wq[CL * t:CL * (t + 1), :]),
            "WK": np.ascontiguousarray(Wk[HD * t:HD * (t + 1), :]),
            "WV": np.ascontiguousarray(Wv[HD * t:HD * (t + 1), :]),
            "WP": np.ascontiguousarray(Wproj[:, CL * t:CL * (t + 1)]),
            "COSW": cosw, "SINW": sinw, "IDENT": ident, "PSWAP": pswap,
            "TRIM": trim, "ONESC": onesc, "AVEC": avec, "BVEC": bvec,
        })

    res = run_bass_kernel_spmd(
        nc, in_maps, core_ids=list(range(8)),
        trace=bool(int(os.environ.get("KERNEL_TRACE", "0"))))
    _CACHE["last_results"] = res

    out = np.zeros((B, S, DIM), dtype=np.float32)
    for c in range(8):
        b = c // 4
        out[b] += res.results[c]["OUT"].T
    return out



# revision 10
# speedup vs baseline: 1.5747x; 1.5747x over previous
"""Causal self-attention (QAT fake-quant weights, RMS-normed q/k, RoPE, GQA)
on 8 Trainium2 NeuronCores.

Sharding: core c = b*4 + t  (b in {0,1} batch, t in {0..3} tensor-parallel).
Per core: 4 q-heads (t*4..t*4+3), 1 kv head (t), Wproj columns [512t, 512t+512).
Each core computes a full [D, S] transposed partial of the output projection
in fp16; the host accumulates the 4 TP partials per batch element in f32.

v2 design (vs v0): keep the PE continuously busy (TRN2 PE p-state ramps
0.65->1.2->2.4 GHz only after ~3us of back-to-back work) and do all matmuls
in fp16 (1 cycle/row incl. transposes; halves SBUF/DMA traffic; DVE gets
2-4x on packed 16-bit SBUF operands).
 - x is pre-converted to fp16 on the host; weights arrive f32, are
   fake-quantized on DVE (scale chain bit-matches the reference), and the
   final mul writes fp16 quantized weights.
 - projections run dt-outer (stationary weight reused across 4 seq chunks,
   4 PSUM banks accumulate in parallel).
 - softmax: probsT = exp(scores - 3) in fp16 (max score ~5.6 so exp fits
   fp16 with margin; the bias cancels in normalization). Row sums come from
   elementwise DVE adds over k-tiles + one gpsimd partition_all_reduce per
   (head, q-chunk) instead of ones-matmuls (saves ~70k PE cycles).
 - yT stays in SBUF (no DRAM spill); the output projection of q-chunk qc is
   interleaved into the attention instruction stream of q-chunk qc+1 so the
   PE never idles while exp (ACT) catches up.
"""

import os
from collections import deque
from contextlib import ExitStack

import numpy as np

import concourse.bass as bass
import concourse.bacc as bacc
import concourse.tile as tile
from concourse import bass_isa, mybir
from concourse.bass_utils import run_bass_kernel_spmd

F32 = mybir.dt.float32
F16N = mybir.dt.float16

DIM = 2048
S = 2048
HD = 128
HL = 4            # local q heads per core
CL = HL * HD      # local head dims (proj contraction)
NB = DIM // 128   # 16 blocks of 128 along a full input-feature axis
MAGIC = float(1.5 * 2 ** 23)
INV31 = float(np.float32(1.0) / np.float32(31.0))
EPS = float(np.finfo(np.float32).eps)
F16_TINY = float(np.finfo(np.float16).tiny)
EXP_BIAS = -3.0

Alu = mybir.AluOpType
Act = mybir.ActivationFunctionType

_CACHE = {}


def build_nc():
    nc = bacc.Bacc("TRN2")

    XT = nc.dram_tensor("XT", [DIM, S], F16N, kind="ExternalInput")
    WQ = nc.dram_tensor("WQ", [CL, DIM], F32, kind="ExternalInput")
    WK = nc.dram_tensor("WK", [HD, DIM], F32, kind="ExternalInput")
    WV = nc.dram_tensor("WV", [HD, DIM], F32, kind="ExternalInput")
    WP = nc.dram_tensor("WP", [DIM, CL], F32, kind="ExternalInput")
    COSW = nc.dram_tensor("COSW", [128, S], F16N, kind="ExternalInput")
    SINW = nc.dram_tensor("SINW", [128, S], F16N, kind="ExternalInput")
    IDENT = nc.dram_tensor("IDENT", [128, 128], F16N, kind="ExternalInput")
    PSWAP = nc.dram_tensor("PSWAP", [128, 128], F16N, kind="ExternalInput")
    TRIM = nc.dram_tensor("TRIM", [128, 128], F16N, kind="ExternalInput")
    ONESC = nc.dram_tensor("ONESC", [128, 1], F16N, kind="ExternalInput")
    AVEC = nc.dram_tensor("AVEC", [1, 8], F32, kind="ExternalInput")
    BVEC = nc.dram_tensor("BVEC", [1, 8], F32, kind="ExternalInput")

    OUT = nc.dram_tensor("OUT", [DIM, S], F16N, kind="ExternalOutput")

    copy_flip = [0]

    def copy_out(dst, src):
        # alternate PSUM->SBUF copies between ACT and DVE
        if copy_flip[0] % 2 == 0:
            nc.scalar.copy(dst, src)
        else:
            nc.vector.tensor_copy(dst, src)
        copy_flip[0] += 1

    with tile.TileContext(nc) as tc, ExitStack() as octx:
        # ---------------- persistent pools ----------------
        pc = octx.enter_context(tc.tile_pool(name="consts", bufs=1))
        pp = octx.enter_context(tc.tile_pool(name="persist", bufs=1))

        ident = pc.tile([128, 128], F16N)
        pswap = pc.tile([128, 128], F16N)
        trim = pc.tile([128, 128], F16N)
        onesc = pc.tile([128, 1], F16N)
        avec = pc.tile([1, 8], F32)
        bvec = pc.tile([1, 8], F32)
        ebias = pc.tile([128, 1], F32)
        nc.vector.memset(ebias[:], EXP_BIAS)

        # persistent operands
        qWkT = pp.tile([128, NB, 128], F16N, tag="qWkT")
        qWvT = pp.tile([128, NB, 128], F16N, tag="qWvT")
        qWQT = [pp.tile([128, NB, 128], F16N, tag=f"qWQT{h}", name=f"qWQT{h}")
                for h in range(HL)]
        qWPT = [pp.tile([128, NB, 128], F16N, tag=f"qWPT{c}", name=f"qWPT{c}")
                for c in range(HL)]
        krot = pp.tile([128, S], F16N, tag="krot")
        qrot = [pp.tile([128, S], F16N, tag=f"qrot{h}", name=f"qrot{h}")
                for h in range(HL)]
        vnat = pp.tile([128, NB, 128], F16N, tag="vnat")  # [s%128, s//128, hd]

        # ============== stack1: weight quant + projections ==============
        es1 = ExitStack()
        px = es1.enter_context(tc.tile_pool(name="p1x", bufs=1))
        pwn = es1.enter_context(tc.tile_pool(name="p1wn", bufs=2))
        pqw = es1.enter_context(tc.tile_pool(name="p1qw", bufs=2))
        psc = es1.enter_context(tc.tile_pool(name="p1sc", bufs=2))
        pwrk = es1.enter_context(tc.tile_pool(name="p1wrk", bufs=2))
        prr = es1.enter_context(tc.tile_pool(name="p1rr", bufs=2))
        pcs = es1.enter_context(tc.tile_pool(name="p1cs", bufs=1))
        ptr = es1.enter_context(tc.tile_pool(name="ps1tr", bufs=2,
                                             space="PSUM"))
        pacc = es1.enter_context(tc.tile_pool(name="ps1acc", bufs=4,
                                              space="PSUM"))
        prow = es1.enter_context(tc.tile_pool(name="ps1row", bufs=1,
                                              space="PSUM"))
        psw = es1.enter_context(tc.tile_pool(name="ps1sw", bufs=1,
                                             space="PSUM"))

        # ---- DMAs up front, in consumption order ----
        nc.sync.dma_start(ident[:], IDENT[:, :])
        nc.sync.dma_start(pswap[:], PSWAP[:, :])
        nc.sync.dma_start(trim[:], TRIM[:, :])
        nc.sync.dma_start(onesc[:], ONESC[:, :])
        nc.sync.dma_start(avec[:], AVEC[:, :])
        nc.sync.dma_start(bvec[:], BVEC[:, :])

        wkn = pwn.tile([128, DIM], F32, tag="wn")
        wvn = pwn.tile([128, DIM], F32, tag="wn")
        for half in range(2):
            hs = slice(half * 1024, (half + 1) * 1024)
            nc.sync.dma_start(wkn[:, hs], WK[:, hs])
        for half in range(2):
            hs = slice(half * 1024, (half + 1) * 1024)
            nc.sync.dma_start(wvn[:, hs], WV[:, hs])
        cosw = pcs.tile([128, S], F16N, tag="cos")
        sinw = pcs.tile([128, S], F16N, tag="sin")
        nc.sync.dma_start(cosw[:], COSW[:, :])
        nc.sync.dma_start(sinw[:], SINW[:, :])

        def quant(wn, nb, qtag, qbufs=None):
            """fp16 fake-quantized copy of natural f32 weight tile wn."""
            amax = psc.tile([128, nb], F32, tag="am")
            nc.vector.tensor_reduce(
                amax[:], wn[:].rearrange("p (b c) -> p b c", c=128),
                axis=mybir.AxisListType.X, op=Alu.max,
                apply_absolute_value=True)
            s0 = psc.tile([128, nb], F32, tag="s0")
            nc.vector.tensor_scalar(s0[:], amax[:], INV31, 1e-12,
                                    Alu.mult, Alu.max)
            s16 = psc.tile([128, nb], F16N, tag="s16")
            nc.vector.tensor_copy(s16[:], s0[:])
            s32 = psc.tile([128, nb], F32, tag="s32")
            nc.vector.tensor_copy(s32[:], s16[:])
            sf = psc.tile([128, nb], F32, tag="sf")
            nc.vector.tensor_scalar_max(sf[:], s32[:], F16_TINY)
            rf = psc.tile([128, nb], F32, tag="rf")
            nc.vector.reciprocal(rf[:], sf[:])

            tt = pwn.tile([128, nb * 128], F32, tag="tt")
            qw = pqw.tile([128, nb * 128], F16N, tag=qtag, bufs=qbufs)
            w3 = wn[:].rearrange("p (b c) -> p b c", c=128)
            t3 = tt[:].rearrange("p (b c) -> p b c", c=128)
            q3 = qw[:].rearrange("p (b c) -> p b c", c=128)
            nc.vector.tensor_mul(t3, w3, rf[:].broadcast_to([128, nb, 128]))
            nc.vector.tensor_scalar(tt[:], tt[:], MAGIC, MAGIC,
                                    Alu.add, Alu.subtract)
            nc.vector.tensor_mul(q3, t3, sf[:].broadcast_to([128, nb, 128]))
            return qw

        def transpose16(qw, dst3):
            """16 [128,128] transposes of qw into dst3 [128, 16, 128]."""
            for g in range(4):
                pt = ptr.tile([128, 4, 128], F16N, tag="tr")
                for j in range(4):
                    nc.tensor.transpose(pt[:, j, :],
                                        qw[:, (4 * g + j) * 128:
                                           (4 * g + j + 1) * 128], ident[:])
                copy_out(dst3[:, 4 * g:4 * g + 4, :]
                         .rearrange("p a b -> p (a b)"),
                         pt[:].rearrange("p a b -> p (a b)"))

        # ---- quantize + transpose K, V, Q ----
        qwk = quant(wkn, NB, "qkv")
        transpose16(qwk, qWkT)
        qwv = quant(wvn, NB, "qkv")
        transpose16(qwv, qWvT)
        for ot in range(HL):
            wt = pwn.tile([128, DIM], F32, tag="wq", name=f"wqn{ot}")
            nc.sync.dma_start(wt[:], WQ[ot * 128:(ot + 1) * 128, :])
            qwq = quant(wt, NB, "qq")
            transpose16(qwq, qWQT[ot])

        def rms_rope_chunk(acc, hm, dst, csl):
            """acc: PSUM [128,512] raw chunk -> dst[:, csl] fp16 normalized
            + roped (gain and 1/sqrt(hd) folded via avec/bvec)."""
            raw = pwrk.tile([128, 512], F16N, tag="raw")
            nc.scalar.copy(raw[:], acc[:])
            qsq = pwrk.tile([128, 512], F16N, tag="qsq")
            nc.vector.tensor_mul(qsq[:], raw[:], raw[:])
            rps = prow.tile([1, 512], F32, tag="row")
            nc.tensor.matmul(rps[:], onesc[:], qsq[:],
                             start=True, stop=True)
            rrow = prr.tile([1, 512], F16N, tag="rr")
            nc.scalar.activation(rrow[:], rps[:], Act.Abs_reciprocal_sqrt,
                                 bias=bvec[0:1, hm:hm + 1],
                                 scale=avec[0:1, hm:hm + 1])
            rb = pwrk.tile([128, 512], F16N, tag="rb", bufs=2)
            nc.gpsimd.partition_broadcast(rb[:], rrow[:])
            qn = pwrk.tile([128, 512], F16N, tag="qn", bufs=2)
            nc.vector.tensor_mul(qn[:], raw[:], rb[:])
            swp = psw.tile([128, 512], F32, tag="sw")
            nc.tensor.matmul(swp[:], pswap[:], qn[:],
                             start=True, stop=True)
            swsb = pwrk.tile([128, 512], F16N, tag="swsb", bufs=2)
            nc.scalar.copy(swsb[:], swp[:])
            u = pwrk.tile([128, 512], F16N, tag="raw")
            nc.vector.tensor_mul(u[:], qn[:], cosw[:, csl])
            w = pwrk.tile([128, 512], F16N, tag="qsq")
            nc.vector.tensor_mul(w[:], swsb[:], sinw[:, csl])
            nc.vector.tensor_add(dst[:, csl], u[:], w[:])

        # ---- projections: 3 passes over x (streamed by seq chunk) ----
        # each pass projects 2 strips; strips: (lhs3, hm, post)
        def post_rot(hm, dst):
            def post(acc, sc):
                rms_rope_chunk(acc, hm, dst, slice(sc * 512, (sc + 1) * 512))
            return post

        def post_v(acc, sc):
            vtr = pwrk.tile([128, 512], F16N, tag="raw")
            nc.scalar.copy(vtr[:], acc[:])
            pt = ptr.tile([128, 4, 128], F16N, tag="tr")
            for j in range(4):
                nc.tensor.transpose(pt[:, j, :],
                                    vtr[:, j * 128:(j + 1) * 128], ident[:])
            copy_out(vnat[:, 4 * sc:4 * sc + 4, :]
                     .rearrange("p a b -> p (a b)"),
                     pt[:].rearrange("p a b -> p (a b)"))

        passes = [
            [(qWkT, post_rot(HL, krot)), (qWvT, post_v)],
            [(qWQT[0], post_rot(0, qrot[0])), (qWQT[1], post_rot(1, qrot[1]))],
            [(qWQT[2], post_rot(2, qrot[2])), (qWQT[3], post_rot(3, qrot[3]))],
        ]
        nxt = [0]
        for pair in passes:
            for sc in range(4):
                ssl = slice(sc * 512, (sc + 1) * 512)
                xts = []
                for dt in range(NB):
                    xt = px.tile([128, 512], F16N, tag="xt", bufs=32,
                                 name=f"xt{nxt[0]}")
                    nxt[0] += 1
                    nc.sync.dma_start(xt[:], XT[dt * 128:(dt + 1) * 128, ssl])
                    xts.append(xt)
                accs = [pacc.tile([128, 512], F32, tag="acc", name=f"acc{i}")
                        for i in range(2)]
                for dt in range(NB):
                    for i, (lhs3, _post) in enumerate(pair):
                        nc.tensor.matmul(accs[i][:], lhs3[:, dt, :],
                                         xts[dt][:], start=(dt == 0),
                                         stop=(dt == NB - 1))
                for i, (_lhs3, post) in enumerate(pair):
                    post(accs[i], sc)

        # ---- Wproj quant + transpose (before stack1 closes) ----
        for grp in range(4):
            qwps = []
            for j in range(4):
                ot = 4 * grp + j
                wn5 = pwn.tile([128, CL], F32, tag="wp")
                nc.sync.dma_start(wn5[:], WP[ot * 128:(ot + 1) * 128, :])
                qwps.append(quant(wn5, 4, "qp", qbufs=5))
            for c in range(4):
                pt = ptr.tile([128, 4, 128], F16N, tag="tr")
                for j in range(4):
                    nc.tensor.transpose(pt[:, j, :],
                                        qwps[j][:, c * 128:(c + 1) * 128],
                                        ident[:])
                copy_out(qWPT[c][:, 4 * grp:4 * grp + 4, :]
                         .rearrange("p a b -> p (a b)"),
                         pt[:].rearrange("p a b -> p (a b)"))

        es1.close()

        # ============== stack2: attention + output projection ==============
        es2 = ExitStack()
        ppr = es2.enter_context(tc.tile_pool(name="p2pr", bufs=8))
        psa = es2.enter_context(tc.tile_pool(name="p2sa", bufs=2))
        pyt = es2.enter_context(tc.tile_pool(name="p2yt", bufs=1))
        pob = es2.enter_context(tc.tile_pool(name="p2ob", bufs=3))
        pscore = es2.enter_context(tc.tile_pool(name="ps2sc", bufs=2,
                                                space="PSUM"))
        pyps = es2.enter_context(tc.tile_pool(name="ps2yp", bufs=1,
                                              space="PSUM"))
        pp6 = es2.enter_context(tc.tile_pool(name="ps2p6", bufs=2,
                                             space="PSUM"))

        yt = [[pyt.tile([128, 512], F16N, tag=f"yt{h}q{qc}",
                        name=f"yt{h}q{qc}") for qc in range(4)]
              for h in range(HL)]

        p6_pending = deque()

        def emit_p6_group():
            ot, qcp = p6_pending.popleft()
            p6 = pp6.tile([128, 512], F32, tag="p6")
            for c in range(4):
                nc.tensor.matmul(p6[:], qWPT[c][:, ot, :], yt[c][qcp][:],
                                 start=(c == 0), stop=(c == 3))
            ob = pob.tile([128, 512], F16N, tag="ob")
            copy_out(ob[:], p6[:])
            nc.sync.dma_start(OUT[ot * 128:(ot + 1) * 128,
                                  qcp * 512:(qcp + 1) * 512], ob[:])

        for qc in range(4):
            nkt = 4 * qc + 4
            qsl0 = qc * 512
            saccs = [psa.tile([128, 512], F16N, tag=f"sa{h}", name=f"sacc{h}")
                     for h in range(HL)]
            ypss = [pyps.tile([128, 512], F32, tag=f"yps{h}", name=f"yps{h}")
                    for h in range(HL)]
            for kt in range(nkt):
                j = kt - 4 * qc
                lo = 0 if j < 0 else 128 * j
                for h in range(HL):
                    scp = pscore.tile([128, 512], F32, tag="sc")
                    nc.tensor.matmul(
                        scp[:, lo:], krot[:, kt * 128:(kt + 1) * 128],
                        qrot[h][:, qsl0 + lo:qsl0 + 512],
                        start=True, stop=True)
                    pr = ppr.tile([128, 512], F16N, tag="pr")
                    nc.scalar.activation(pr[:, lo:], scp[:, lo:], Act.Exp,
                                         bias=ebias[:])
                    if j >= 0:
                        nc.vector.tensor_mul(pr[:, lo:lo + 128],
                                             pr[:, lo:lo + 128], trim[:])
                    nc.tensor.matmul(ypss[h][:, lo:], vnat[:, kt, :],
                                     pr[:, lo:], start=(kt == 0),
                                     stop=(kt == nkt - 1))
                    if kt == 0:
                        nc.vector.tensor_copy(saccs[h][:], pr[:])
                    else:
                        nc.vector.tensor_add(saccs[h][:, lo:],
                                             saccs[h][:, lo:], pr[:, lo:])
                for _ in range(2):
                    if p6_pending:
                        emit_p6_group()
            for h in range(HL):
                sret = psa.tile([128, 512], F32, tag="sret")
                nc.gpsimd.partition_all_reduce(sret[:], saccs[h][:], 128,
                                               bass_isa.ReduceOp.add)
                rs = psa.tile([128, 512], F32, tag="rs")
                nc.vector.reciprocal(rs[:], sret[:])
                nc.vector.tensor_mul(yt[h][qc][:], ypss[h][:], rs[:])
            p6_pending.extend((ot, qc) for ot in range(NB))
        while p6_pending:
            emit_p6_group()
        es2.close()

    nc.compile()
    return nc


# --------------------------------------------------------------------------
# host side
# --------------------------------------------------------------------------

def _host_consts():
    inv_freq = 1.0 / (10000.0 ** (np.arange(0, HD, 2, dtype=np.float32)
                                  / np.float32(HD)))
    freqs = np.outer(np.arange(S, dtype=np.float32),
                     inv_freq).astype(np.float32)       # [S, 64]
    cosT = np.cos(freqs).astype(np.float32).T           # [64, S]
    sinT = np.sin(freqs).astype(np.float32).T
    cosw = np.ascontiguousarray(
        np.concatenate([cosT, cosT], axis=0)).astype(np.float16)
    sinw = np.ascontiguousarray(
        np.concatenate([sinT, -sinT], axis=0)).astype(np.float16)
    ident = np.eye(128, dtype=np.float16)
    pswap = np.zeros((128, 128), dtype=np.float16)
    pswap[:64, 64:] = np.eye(64)
    pswap[64:, :64] = np.eye(64)
    trim = (np.arange(128)[:, None] <= np.arange(128)[None, :]) \
        .astype(np.float16)                             # allow k <= q
    onesc = np.ones((128, 1), dtype=np.float16)
    return cosw, sinw, ident, pswap, trim, onesc


def kernel(x, Wq, Wk, Wv, Wproj, q_gain):
    x = np.asarray(x, dtype=np.float32)
    Wq = np.asarray(Wq, dtype=np.float32)
    Wk = np.asarray(Wk, dtype=np.float32)
    Wv = np.asarray(Wv, dtype=np.float32)
    Wproj = np.asarray(Wproj, dtype=np.float32)
    q_gain = np.asarray(q_gain, dtype=np.float32)
    B = x.shape[0]

    if "nc" not in _CACHE:
        _CACHE["nc"] = build_nc()
    nc = _CACHE["nc"]

    cosw, sinw, ident, pswap, trim, onesc = _host_consts()

    in_maps = []
    for c in range(8):
        b, t = divmod(c, 4)
        g = q_gain[4 * t:4 * t + 4].astype(np.float64)
        avec = np.zeros((1, 8), dtype=np.float32)
        bvec = np.zeros((1, 8), dtype=np.float32)
        avec[0, :4] = (1.0 / g ** 2).astype(np.float32)
        avec[0, 4] = np.float32(1.0 / 128.0)
        bvec[0, :4] = (128.0 * EPS / g ** 2).astype(np.float32)
        bvec[0, 4] = np.float32(EPS)
        in_maps.append({
            "XT": np.ascontiguousarray(x[b].T).astype(np.float16),
            "WQ": np.ascontiguousarray(Wq[CL * t:CL * (t + 1), :]),
            "WK": np.ascontiguousarray(Wk[HD * t:HD * (t + 1), :]),
            "WV": np.ascontiguousarray(Wv[HD * t:HD * (t + 1), :]),
            "WP": np.ascontiguousarray(Wproj[:, CL * t:CL * (t + 1)]),
            "COSW": cosw, "SINW": sinw, "IDENT": ident, "PSWAP": pswap,
            "TRIM": trim, "ONESC": onesc, "AVEC": avec, "BVEC": bvec,
        })

    res = run_bass_kernel_spmd(
        nc, in_maps, core_ids=list(range(8)),
        trace=bool(int(os.environ.get("KERNEL_TRACE", "0"))))
    _CACHE["last_results"] = res

    out = np.zeros((B, S, DIM), dtype=np.float32)
    for c in range(8):
        b = c // 4
        out[b] += res.results[c]["OUT"].T.astype(np.float32)
    return out


# revision 18
# speedup vs baseline: 1.7033x; 1.0817x over previous
"""Causal self-attention (QAT fake-quant weights, RMS-normed q/k, RoPE, GQA)
on 8 Trainium2 NeuronCores.

Sharding: core c = b*4 + t  (b in {0,1} batch, t in {0..3} tensor-parallel).
Per core: 4 q-heads (t*4..t*4+3), 1 kv head (t), Wproj columns [512t, 512t+512).
Each core computes a full [D, S] transposed partial of the output projection
in fp16; the host accumulates the 4 TP partials per batch element in f32.

v2 design (vs v0): keep the PE continuously busy (TRN2 PE p-state ramps
0.65->1.2->2.4 GHz only after ~3us of back-to-back work) and do all matmuls
in fp16 (1 cycle/row incl. transposes; halves SBUF/DMA traffic; DVE gets
2-4x on packed 16-bit SBUF operands).
 - x is pre-converted to fp16 on the host; weights arrive f32, are
   fake-quantized on DVE (scale chain bit-matches the reference), and the
   final mul writes fp16 quantized weights.
 - projections run dt-outer (stationary weight reused across 4 seq chunks,
   4 PSUM banks accumulate in parallel).
 - softmax: probsT = exp(scores - 3) in fp16 (max score ~5.6 so exp fits
   fp16 with margin; the bias cancels in normalization). Row sums come from
   elementwise DVE adds over k-tiles + one gpsimd partition_all_reduce per
   (head, q-chunk) instead of ones-matmuls (saves ~70k PE cycles).
 - yT stays in SBUF (no DRAM spill); the output projection of q-chunk qc is
   interleaved into the attention instruction stream of q-chunk qc+1 so the
   PE never idles while exp (ACT) catches up.
"""

import os
from collections import deque
from contextlib import ExitStack

import numpy as np

import concourse.bass as bass
import concourse.bacc as bacc
import concourse.tile as tile
from concourse import bass_isa, mybir
from concourse.bass_utils import run_bass_kernel_spmd

F32 = mybir.dt.float32
F16N = mybir.dt.float16

DIM = 2048
S = 2048
HD = 128
HL = 4            # local q heads per core
CL = HL * HD      # local head dims (proj contraction)
NB = DIM // 128   # 16 blocks of 128 along a full input-feature axis
MAGIC = float(1.5 * 2 ** 23)
INV31 = float(np.float32(1.0) / np.float32(31.0))
EPS = float(np.finfo(np.float32).eps)
F16_TINY = float(np.finfo(np.float16).tiny)
EXP_BIAS = -3.0

Alu = mybir.AluOpType
Act = mybir.ActivationFunctionType

_CACHE = {}


def build_nc():
    nc = bacc.Bacc("TRN2")

    XT = nc.dram_tensor("XT", [DIM, S], F16N, kind="ExternalInput")
    WQ = nc.dram_tensor("WQ", [CL, DIM], F32, kind="ExternalInput")
    WK = nc.dram_tensor("WK", [HD, DIM], F32, kind="ExternalInput")
    WV = nc.dram_tensor("WV", [HD, DIM], F32, kind="ExternalInput")
    WP = nc.dram_tensor("WP", [DIM, CL], F32, kind="ExternalInput")
    COSW = nc.dram_tensor("COSW", [128, S], F16N, kind="ExternalInput")
    SINW = nc.dram_tensor("SINW", [128, S], F16N, kind="ExternalInput")
    IDENT = nc.dram_tensor("IDENT", [128, 128], F16N, kind="ExternalInput")
    PSWAP = nc.dram_tensor("PSWAP", [128, 128], F16N, kind="ExternalInput")
    TRIM = nc.dram_tensor("TRIM", [128, 128], F16N, kind="ExternalInput")
    ONESC = nc.dram_tensor("ONESC", [128, 1], F16N, kind="ExternalInput")
    AVEC = nc.dram_tensor("AVEC", [1, 8], F32, kind="ExternalInput")
    BVEC = nc.dram_tensor("BVEC", [1, 8], F32, kind="ExternalInput")

    OUT = nc.dram_tensor("OUT", [DIM, S], F16N, kind="ExternalOutput")

    copy_flip = [0]

    def copy_out(dst, src):
        # alternate PSUM->SBUF copies between ACT and DVE
        if copy_flip[0] % 2 == 0:
            nc.scalar.copy(dst, src)
        else:
            nc.vector.tensor_copy(dst, src)
        copy_flip[0] += 1

    with tile.TileContext(nc) as tc, ExitStack() as octx:
        # ---------------- persistent pools ----------------
        pc = octx.enter_context(tc.tile_pool(name="consts", bufs=1))
        pp = octx.enter_context(tc.tile_pool(name="persist", bufs=1))

        ident = pc.tile([128, 128], F16N)
        pswap = pc.tile([128, 128], F16N)
        trim = pc.tile([128, 128], F16N)
        onesc = pc.tile([128, 1], F16N)
        avec = pc.tile([1, 8], F32)
        bvec = pc.tile([1, 8], F32)
        ebias = pc.tile([128, 1], F32)
        nc.vector.memset(ebias[:], EXP_BIAS)

        # persistent operands
        qWkT = pp.tile([128, NB, 128], F16N, tag="qWkT")
        qWvT = pp.tile([128, NB, 128], F16N, tag="qWvT")
        qWQT = [pp.tile([128, NB, 128], F16N, tag=f"qWQT{h}", name=f"qWQT{h}")
                for h in range(HL)]
        qWPT = [pp.tile([128, NB, 128], F16N, tag=f"qWPT{c}", name=f"qWPT{c}")
                for c in range(HL)]
        krot = pp.tile([128, S], F16N, tag="krot")
        qrot = [pp.tile([128, S], F16N, tag=f"qrot{h}", name=f"qrot{h}")
                for h in range(HL)]
        vnat = pp.tile([128, NB, 128], F16N, tag="vnat")  # [s%128, s//128, hd]

        # ============== stack1: weight quant + projections ==============
        es1 = ExitStack()
        px = es1.enter_context(tc.tile_pool(name="p1x", bufs=1))
        pwn = es1.enter_context(tc.tile_pool(name="p1wn", bufs=2))
        pqw = es1.enter_context(tc.tile_pool(name="p1qw", bufs=2))
        psc = es1.enter_context(tc.tile_pool(name="p1sc", bufs=2))
        pwrk = es1.enter_context(tc.tile_pool(name="p1wrk", bufs=2))
        prr = es1.enter_context(tc.tile_pool(name="p1rr", bufs=2))
        pcs = es1.enter_context(tc.tile_pool(name="p1cs", bufs=1))
        ptr = es1.enter_context(tc.tile_pool(name="ps1tr", bufs=2,
                                             space="PSUM"))
        pacc = es1.enter_context(tc.tile_pool(name="ps1acc", bufs=4,
                                              space="PSUM"))
        prow = es1.enter_context(tc.tile_pool(name="ps1row", bufs=1,
                                              space="PSUM"))
        psw = es1.enter_context(tc.tile_pool(name="ps1sw", bufs=1,
                                             space="PSUM"))

        # ---- DMAs up front, in consumption order (sync queue issues one
        # dma_start every ~600ns, so the order IS the startup schedule) ----
        wkn = pwn.tile([128, DIM], F32, tag="wn")
        wvn = pwn.tile([128, DIM], F32, tag="wn")
        for half in range(2):
            hs = slice(half * 1024, (half + 1) * 1024)
            nc.sync.dma_start(wkn[:, hs], WK[:, hs])
        for half in range(2):
            hs = slice(half * 1024, (half + 1) * 1024)
            nc.sync.dma_start(wvn[:, hs], WV[:, hs])
        nc.sync.dma_start(ident[:], IDENT[:, :])
        nc.sync.dma_start(pswap[:], PSWAP[:, :])
        nc.sync.dma_start(trim[:], TRIM[:, :])
        nc.sync.dma_start(onesc[:], ONESC[:, :])
        nc.sync.dma_start(avec[:], AVEC[:, :])
        nc.sync.dma_start(bvec[:], BVEC[:, :])
        xts = []
        for dt in range(NB):
            xt = px.tile([128, S], F16N, tag=f"xt{dt}", name=f"xt{dt}")
            nc.sync.dma_start(xt[:], XT[dt * 128:(dt + 1) * 128, :])
            xts.append(xt)
        cosw = pcs.tile([128, S], F16N, tag="cos")
        sinw = pcs.tile([128, S], F16N, tag="sin")
        nc.sync.dma_start(cosw[:], COSW[:, :])
        nc.sync.dma_start(sinw[:], SINW[:, :])

        def quant(wn, nb, qtag, qbufs=None):
            """fp16 fake-quantized copy of natural f32 weight tile wn.
            Chain spread over gpsimd (amax) / DVE (scales, round, final mul)
            / ACT (per-block w*rf) so tiles pipeline at DVE-chain cadence."""
            amax = psc.tile([128, nb], F32, tag="am")
            nc.vector.tensor_reduce(
                amax[:], wn[:].rearrange("p (b c) -> p b c", c=128),
                axis=mybir.AxisListType.X, op=Alu.max,
                apply_absolute_value=True)
            s0 = psc.tile([128, nb], F32, tag="s0")
            nc.vector.tensor_scalar(s0[:], amax[:], INV31, 1e-12,
                                    Alu.mult, Alu.max)
            s16 = psc.tile([128, nb], F16N, tag="s16")
            nc.vector.tensor_copy(s16[:], s0[:])
            s32 = psc.tile([128, nb], F32, tag="s32")
            nc.vector.tensor_copy(s32[:], s16[:])
            sf = psc.tile([128, nb], F32, tag="sf")
            nc.vector.tensor_scalar_max(sf[:], s32[:], F16_TINY)
            rf = psc.tile([128, nb], F32, tag="rf")
            nc.vector.reciprocal(rf[:], sf[:])

            tt = pwn.tile([128, nb * 128], F32, tag="tt", bufs=1)
            qw = pqw.tile([128, nb * 128], F16N, tag=qtag, bufs=qbufs)
            t3 = tt[:].rearrange("p (b c) -> p b c", c=128)
            q3 = qw[:].rearrange("p (b c) -> p b c", c=128)
            for b in range(nb):
                nc.scalar.activation(t3[:, b, :],
                                     wn[:, b * 128:(b + 1) * 128], Act.Copy,
                                     scale=rf[:, b:b + 1])
            nc.vector.tensor_scalar(tt[:], tt[:], MAGIC, MAGIC,
                                    Alu.add, Alu.subtract)
            nc.vector.tensor_mul(q3, t3, sf[:].broadcast_to([128, nb, 128]))
            return qw

        def transpose16(qw, dst3):
            """16 [128,128] transposes of qw into dst3 [128, 16, 128]."""
            for g in range(4):
                pt = ptr.tile([128, 4, 128], F16N, tag="tr")
                for j in range(4):
                    nc.tensor.transpose(pt[:, j, :],
                                        qw[:, (4 * g + j) * 128:
                                           (4 * g + j + 1) * 128], ident[:])
                copy_out(dst3[:, 4 * g:4 * g + 4, :]
                         .rearrange("p a b -> p (a b)"),
                         pt[:].rearrange("p a b -> p (a b)"))

        # ---- quantize + transpose K, V ----
        qwk = quant(wkn, NB, "qkv")
        transpose16(qwk, qWkT)
        qwv = quant(wvn, NB, "qkv")
        transpose16(qwv, qWvT)

        def rms_rope_chunk(acc, hm, dst, csl):
            """acc: PSUM [128,512] raw chunk -> dst[:, csl] fp16 normalized
            + roped (gain and 1/sqrt(hd) folded via avec/bvec)."""
            raw = pwrk.tile([128, 512], F16N, tag="raw")
            nc.scalar.copy(raw[:], acc[:])
            qsq = pwrk.tile([128, 512], F16N, tag="qsq")
            nc.vector.tensor_mul(qsq[:], raw[:], raw[:])
            rps = prow.tile([1, 512], F32, tag="row")
            nc.tensor.matmul(rps[:], onesc[:], qsq[:],
                             start=True, stop=True)
            rrow = prr.tile([1, 512], F16N, tag="rr")
            nc.scalar.activation(rrow[:], rps[:], Act.Abs_reciprocal_sqrt,
                                 bias=bvec[0:1, hm:hm + 1],
                                 scale=avec[0:1, hm:hm + 1])
            rb = pwrk.tile([128, 512], F16N, tag="rb", bufs=2)
            nc.gpsimd.partition_broadcast(rb[:], rrow[:])
            qn = pwrk.tile([128, 512], F16N, tag="qn", bufs=2)
            nc.vector.tensor_mul(qn[:], raw[:], rb[:])
            swp = psw.tile([128, 512], F32, tag="sw")
            nc.tensor.matmul(swp[:], pswap[:], qn[:],
                             start=True, stop=True)
            swsb = pwrk.tile([128, 512], F16N, tag="swsb", bufs=2)
            nc.scalar.copy(swsb[:], swp[:])
            u = pwrk.tile([128, 512], F16N, tag="raw")
            nc.vector.tensor_mul(u[:], qn[:], cosw[:, csl])
            w = pwrk.tile([128, 512], F16N, tag="qsq")
            nc.vector.tensor_mul(w[:], swsb[:], sinw[:, csl])
            nc.vector.tensor_add(dst[:, csl], u[:], w[:])

        # ---- projections: strip-major over resident x; dt-outer so the
        # stationary weight tile is reused by 4 consecutive matmuls ----
        def post_rot(hm, dst):
            def post(acc, sc):
                rms_rope_chunk(acc, hm, dst, slice(sc * 512, (sc + 1) * 512))
            return post

        def post_v(acc, sc):
            vtr = pwrk.tile([128, 512], F16N, tag="raw")
            nc.scalar.copy(vtr[:], acc[:])
            pt = ptr.tile([128, 4, 128], F16N, tag="tr")
            for j in range(4):
                nc.tensor.transpose(pt[:, j, :],
                                    vtr[:, j * 128:(j + 1) * 128], ident[:])
            copy_out(vnat[:, 4 * sc:4 * sc + 4, :]
                     .rearrange("p a b -> p (a b)"),
                     pt[:].rearrange("p a b -> p (a b)"))

        def strip(lhs3, post):
            accs = [pacc.tile([128, 512], F32, tag="acc", name=f"acc{i}")
                    for i in range(4)]
            for dt in range(NB):
                for sc in range(4):
                    nc.tensor.matmul(accs[sc][:], lhs3[:, dt, :],
                                     xts[dt][:, sc * 512:(sc + 1) * 512],
                                     start=(dt == 0), stop=(dt == NB - 1))
            for sc in range(4):
                post(accs[sc], sc)

        strip(qWkT, post_rot(HL, krot))
        strip(qWvT, post_v)
        for h in range(HL):
            wt = pwn.tile([128, DIM], F32, tag="wq", bufs=1, name=f"wqn{h}")
            nc.sync.dma_start(wt[:], WQ[h * 128:(h + 1) * 128, :])
            qwq = quant(wt, NB, "qq", qbufs=1)
            transpose16(qwq, qWQT[h])
            strip(qWQT[h], post_rot(h, qrot[h]))

        # ---- Wproj quant + transpose (before stack1 closes) ----
        for grp in range(4):
            qwps = []
            for j in range(4):
                ot = 4 * grp + j
                wn5 = pwn.tile([128, CL], F32, tag="wp")
                nc.sync.dma_start(wn5[:], WP[ot * 128:(ot + 1) * 128, :])
                qwps.append(quant(wn5, 4, "qp", qbufs=5))
            for c in range(4):
                pt = ptr.tile([128, 4, 128], F16N, tag="tr")
                for j in range(4):
                    nc.tensor.transpose(pt[:, j, :],
                                        qwps[j][:, c * 128:(c + 1) * 128],
                                        ident[:])
                copy_out(qWPT[c][:, 4 * grp:4 * grp + 4, :]
                         .rearrange("p a b -> p (a b)"),
                         pt[:].rearrange("p a b -> p (a b)"))

        es1.close()

        # ============== stack2: attention + output projection ==============
        es2 = ExitStack()
        ppr = es2.enter_context(tc.tile_pool(name="p2pr", bufs=8))
        psa = es2.enter_context(tc.tile_pool(name="p2sa", bufs=2))
        pyt = es2.enter_context(tc.tile_pool(name="p2yt", bufs=1))
        pob = es2.enter_context(tc.tile_pool(name="p2ob", bufs=3))
        pscore = es2.enter_context(tc.tile_pool(name="ps2sc", bufs=3,
                                                space="PSUM"))
        pyps = es2.enter_context(tc.tile_pool(name="ps2yp", bufs=1,
                                              space="PSUM"))
        pp6 = es2.enter_context(tc.tile_pool(name="ps2p6", bufs=1,
                                             space="PSUM"))

        yt = [[pyt.tile([128, 512], F16N, tag=f"yt{h}q{qc}",
                        name=f"yt{h}q{qc}") for qc in range(4)]
              for h in range(HL)]

        p6_pending = deque()

        def emit_p6_group(pool=None, tag="p6"):
            ot, qcp = p6_pending.popleft()
            p6 = (pool or pp6).tile([128, 512], F32, tag=tag, name="p6")
            for c in range(4):
                nc.tensor.matmul(p6[:], qWPT[c][:, ot, :], yt[c][qcp][:],
                                 start=(c == 0), stop=(c == 3))
            ob = pob.tile([128, 512], F16N, tag="ob")
            copy_out(ob[:], p6[:])
            nc.sync.dma_start(OUT[ot * 128:(ot + 1) * 128,
                                  qcp * 512:(qcp + 1) * 512], ob[:])

        for qc in range(4):
            nkt = 4 * qc + 4
            qsl0 = qc * 512
            saccs = [psa.tile([128, 512], F16N, tag=f"sa{h}", name=f"sacc{h}")
                     for h in range(HL)]
            ypss = [pyps.tile([128, 512], F32, tag=f"yps{h}", name=f"yps{h}")
                    for h in range(HL)]
            for kt in range(nkt):
                j = kt - 4 * qc
                lo = 0 if j < 0 else 128 * j
                # 4 scores sharing the stationary krot tile, then the 4
                # exps, then 4 PVs sharing stationary vnat[kt]
                prs = []
                for h in range(HL):
                    scp = pscore.tile([128, 512], F32, tag="sc",
                                      name=f"scp{h}")
                    nc.tensor.matmul(
                        scp[:, lo:], krot[:, kt * 128:(kt + 1) * 128],
                        qrot[h][:, qsl0 + lo:qsl0 + 512],
                        start=True, stop=True)
                    pr = ppr.tile([128, 512], F16N, tag="pr",
                                  name=f"pr{h}")
                    nc.scalar.activation(pr[:, lo:], scp[:, lo:], Act.Exp,
                                         bias=ebias[:])
                    if j >= 0:
                        nc.vector.tensor_mul(pr[:, lo:lo + 128],
                                             pr[:, lo:lo + 128], trim[:])
                    prs.append(pr)
                for h in range(HL):
                    nc.tensor.matmul(ypss[h][:, lo:], vnat[:, kt, :],
                                     prs[h][:, lo:], start=(kt == 0),
                                     stop=(kt == nkt - 1))
                for h in range(HL):
                    if kt == 0:
                        nc.vector.tensor_copy(saccs[h][:], prs[h][:])
                    else:
                        nc.vector.tensor_add(saccs[h][:, lo:],
                                             saccs[h][:, lo:], prs[h][:, lo:])
                for _ in range(2):
                    if p6_pending:
                        emit_p6_group()
            for h in range(HL):
                sret = psa.tile([128, 512], F32, tag="sret")
                nc.gpsimd.partition_all_reduce(sret[:], saccs[h][:], 128,
                                               bass_isa.ReduceOp.add)
                rs = psa.tile([128, 512], F32, tag="rs")
                nc.vector.reciprocal(rs[:], sret[:])
                nc.vector.tensor_mul(yt[h][qc][:], ypss[h][:], rs[:])
            p6_pending.extend((ot, qc) for ot in range(NB))
        while p6_pending:
            # attention is done: reuse the idle score banks to pipeline
            emit_p6_group(pool=pscore, tag="sc")
        es2.close()

    nc.compile()
    return nc


# --------------------------------------------------------------------------
# host side
# --------------------------------------------------------------------------

def _host_consts():
    inv_freq = 1.0 / (10000.0 ** (np.arange(0, HD, 2, dtype=np.float32)
                                  / np.float32(HD)))
    freqs = np.outer(np.arange(S, dtype=np.float32),
                     inv_freq).astype(np.float32)       # [S, 64]
    cosT = np.cos(freqs).astype(np.float32).T           # [64, S]
    sinT = np.sin(freqs).astype(np.float32).T
    cosw = np.ascontiguousarray(
        np.concatenate([cosT, cosT], axis=0)).astype(np.float16)
    sinw = np.ascontiguousarray(
        np.concatenate([sinT, -sinT], axis=0)).astype(np.float16)
    ident = np.eye(128, dtype=np.float16)
    pswap = np.zeros((128, 128), dtype=np.float16)
    pswap[:64, 64:] = np.eye(64)
    pswap[64:, :64] = np.eye(64)
    trim = (np.arange(128)[:, None] <= np.arange(128)[None, :]) \
        .astype(np.float16)                             # allow k <= q
    onesc = np.ones((128, 1), dtype=np.float16)
    return cosw, sinw, ident, pswap, trim, onesc


def kernel(x, Wq, Wk, Wv, Wproj, q_gain):
    x = np.asarray(x, dtype=np.float32)
    Wq = np.asarray(Wq, dtype=np.float32)
    Wk = np.asarray(Wk, dtype=np.float32)
    Wv = np.asarray(Wv, dtype=np.float32)
    Wproj = np.asarray(Wproj, dtype=np.float32)
    q_gain = np.asarray(q_gain, dtype=np.float32)
    B = x.shape[0]

    if "nc" not in _CACHE:
        _CACHE["nc"] = build_nc()
    nc = _CACHE["nc"]

    cosw, sinw, ident, pswap, trim, onesc = _host_consts()

    in_maps = []
    for c in range(8):
        b, t = divmod(c, 4)
        g = q_gain[4 * t:4 * t + 4].astype(np.float64)
        avec = np.zeros((1, 8), dtype=np.float32)
        bvec = np.zeros((1, 8), dtype=np.float32)
        avec[0, :4] = (1.0 / g ** 2).astype(np.float32)
        avec[0, 4] = np.float32(1.0 / 128.0)
        bvec[0, :4] = (128.0 * EPS / g ** 2).astype(np.float32)
        bvec[0, 4] = np.float32(EPS)
        in_maps.append({
            "XT": np.ascontiguousarray(x[b].T).astype(np.float16),
            "WQ": np.ascontiguousarray(Wq[CL * t:CL * (t + 1), :]),
            "WK": np.ascontiguousarray(Wk[HD * t:HD * (t + 1), :]),
            "WV": np.ascontiguousarray(Wv[HD * t:HD * (t + 1), :]),
            "WP": np.ascontiguousarray(Wproj[:, CL * t:CL * (t + 1)]),
            "COSW": cosw, "SINW": sinw, "IDENT": ident, "PSWAP": pswap,
            "TRIM": trim, "ONESC": onesc, "AVEC": avec, "BVEC": bvec,
        })

    res = run_bass_kernel_spmd(
        nc, in_maps, core_ids=list(range(8)),
        trace=bool(int(os.environ.get("KERNEL_TRACE", "0"))))
    _CACHE["last_results"] = res

    out = np.zeros((B, S, DIM), dtype=np.float32)
    for c in range(8):
        b = c // 4
        out[b] += res.results[c]["OUT"].T.astype(np.float32)
    return out
